# revision 4
# baseline (speedup 1.0000x reference)
"""DenseGrid multi-LOD bilinear embedding lookup on 8 Trainium2 NeuronCores.

Strategy: data-parallel over points (8-way shard). Grids are host-expanded
into per-cell "quad" tables holding the 4 bilinear corners in pre-differenced
form [g12-g11, g22-g21, g11, g21] so the device does a single 64B (fp32) /
32B (fp16) indirect-DMA gather per (point, LOD) and a 2-stage Horner lerp:
    r_i = g_i1 + fx * d_i          (i = rows y1, y1+1)
    out = r_1 + fy * (r_2 - r_1)
"""
import numpy as np
import concourse.bacc as bacc
import concourse.bass as bass
import concourse.mybir as mybir
import concourse.tile as tile
from concourse.bass_utils import run_bass_kernel_spmd

BASE_LOD = 4
NUM_LODS = 8
FEAT = 4
LODS = [2 ** L for L in range(BASE_LOD, BASE_LOD + NUM_LODS)]
N_POINTS = 2_000_000
N_CORES = 8
P = 128
PPP = 2048            # points per partition (per core)
CN = 256              # points per partition per chunk
CHUNKS = PPP // CN
PTS_PER_CORE = P * PPP

QUAD_DT = np.float16  # table dtype (fp32 also supported)
_BIR_QDT = {np.float16: mybir.dt.float16, np.float32: mybir.dt.float32}

_cache = {}


def _build_program(qdt):
    bir_qdt = _BIR_QDT[qdt]
    nc = bacc.Bacc(None, target_bir_lowering=False)
    with tile.TileContext(nc) as tc:
        with tc.tile_pool(name="dram", bufs=1, space="DRAM") as dram, \
             tc.tile_pool(name="io", bufs=2) as io, \
             tc.tile_pool(name="qp", bufs=3) as qp, \
             tc.tile_pool(name="wk", bufs=2) as wk:
            x_d = dram.tile([P, PPP * 2], mybir.dt.float32, kind="ExternalInput")
            q_d = [dram.tile([res * res, 16], bir_qdt, kind="ExternalInput",
                             name=f"quad_{li}")
                   for li, res in enumerate(LODS)]
            out_d = dram.tile([P, PPP * NUM_LODS * FEAT], mybir.dt.float32,
                              kind="ExternalOutput")

            for c in range(CHUNKS):
                xt = io.tile([P, CN * 2], mybir.dt.float32, tag="x")
                nc.sync.dma_start(out=xt[:], in_=x_d[:, c * CN * 2:(c + 1) * CN * 2])
                x3 = xt[:].rearrange("p (n two) -> p n two", two=2)
                ot = io.tile([P, CN * NUM_LODS * FEAT], mybir.dt.float32, tag="o")
                o3 = ot[:].rearrange("p (n f) -> p n f", f=NUM_LODS * FEAT)

                for l, res in enumerate(LODS):
                    # --- index / fraction compute ---
                    xs = wk.tile([P, CN], mybir.dt.float32, tag="xs")
                    ys = wk.tile([P, CN], mybir.dt.float32, tag="ys")
                    nc.scalar.activation(out=xs[:], in_=x3[:, :, 0],
                                         func=mybir.ActivationFunctionType.Copy,
                                         scale=float(res - 1))
                    nc.scalar.activation(out=ys[:], in_=x3[:, :, 1],
                                         func=mybir.ActivationFunctionType.Copy,
                                         scale=float(res - 1))
                    # floor via round-nearest int convert of (min(xs,hi) - 0.5);
                    # ties land only on exact-integer coords where the lerp
                    # result is unchanged (fx becomes 1.0 instead of 0.0).
                    hi = float(res - 1) - 1e-5
                    xc = wk.tile([P, CN], mybir.dt.float32, tag="xc")
                    yc = wk.tile([P, CN], mybir.dt.float32, tag="yc")
                    nc.vector.tensor_scalar(out=xc[:], in0=xs[:], scalar1=hi,
                                            scalar2=0.5, op0=mybir.AluOpType.min,
                                            op1=mybir.AluOpType.subtract)
                    nc.vector.tensor_scalar(out=yc[:], in0=ys[:], scalar1=hi,
                                            scalar2=0.5, op0=mybir.AluOpType.min,
                                            op1=mybir.AluOpType.subtract)
                    x1i = wk.tile([P, CN], mybir.dt.int32, tag="x1i")
                    y1i = wk.tile([P, CN], mybir.dt.int32, tag="y1i")
                    nc.vector.tensor_copy(out=x1i[:], in_=xc[:])
                    nc.vector.tensor_copy(out=y1i[:], in_=yc[:])
                    x1f = wk.tile([P, CN], mybir.dt.float32, tag="x1f")
                    y1f = wk.tile([P, CN], mybir.dt.float32, tag="y1f")
                    nc.vector.tensor_copy(out=x1f[:], in_=x1i[:])
                    nc.vector.tensor_copy(out=y1f[:], in_=y1i[:])
                    fx = wk.tile([P, CN], mybir.dt.float32, tag="fx")
                    fy = wk.tile([P, CN], mybir.dt.float32, tag="fy")
                    nc.vector.tensor_sub(out=fx[:], in0=xs[:], in1=x1f[:])
                    nc.vector.tensor_sub(out=fy[:], in0=ys[:], in1=y1f[:])
                    idf = wk.tile([P, CN], mybir.dt.float32, tag="idf")
                    nc.vector.scalar_tensor_tensor(
                        out=idf[:], in0=y1f[:], scalar=float(res), in1=x1f[:],
                        op0=mybir.AluOpType.mult, op1=mybir.AluOpType.add)
                    idx = wk.tile([P, CN], mybir.dt.int32, tag="idx")
                    nc.vector.tensor_copy(out=idx[:], in_=idf[:])

                    # --- gather quads ---
                    # HW indirect DMA uses ONE offset per partition, so issue
                    # one call per point-column (each gathers 128 quads).
                    qt = qp.tile([P, CN * 16], bir_qdt, tag="q")
                    for j in range(CN):
                        nc.gpsimd.indirect_dma_start(
                            out=qt[:, j * 16:(j + 1) * 16], out_offset=None,
                            in_=q_d[l][:],
                            in_offset=bass.IndirectOffsetOnAxis(
                                ap=idx[:, j:j + 1], axis=0))
                    q4 = qt[:].rearrange("p (n c f) -> p n c f", c=4, f=4)

                    # --- Horner bilinear combine ---
                    # quad layout: [d1, d2, g11, g21]
                    fxb = fx[:].unsqueeze(2).unsqueeze(3).broadcast_to([P, CN, 2, 4])
                    m = wk.tile([P, CN * 8], mybir.dt.float32, tag="m")
                    m4 = m[:].rearrange("p (n c f) -> p n c f", c=2, f=4)
                    nc.vector.tensor_mul(out=m4, in0=q4[:, :, 0:2, :], in1=fxb)
                    r = wk.tile([P, CN * 8], mybir.dt.float32, tag="r")
                    r4 = r[:].rearrange("p (n c f) -> p n c f", c=2, f=4)
                    nc.vector.tensor_add(out=r4, in0=m4, in1=q4[:, :, 2:4, :])
                    dy = wk.tile([P, CN * 4], mybir.dt.float32, tag="dy")
                    dy3 = dy[:].rearrange("p (n f) -> p n f", f=4)
                    nc.vector.tensor_sub(out=dy3, in0=r4[:, :, 1, :], in1=r4[:, :, 0, :])
                    fyb = fy[:].unsqueeze(2).broadcast_to([P, CN, 4])
                    my = wk.tile([P, CN * 4], mybir.dt.float32, tag="my")
                    my3 = my[:].rearrange("p (n f) -> p n f", f=4)
                    nc.vector.tensor_mul(out=my3, in0=dy3, in1=fyb)
                    nc.vector.tensor_add(out=o3[:, :, l * FEAT:(l + 1) * FEAT],
                                         in0=my3, in1=r4[:, :, 0, :])

                nc.sync.dma_start(
                    out=out_d[:, c * CN * NUM_LODS * FEAT:(c + 1) * CN * NUM_LODS * FEAT],
                    in_=ot[:])
    nc.compile()
    names = {"x": x_d.name, "q": [t.name for t in q_d], "out": out_d.name}
    return nc, names


def _quad_table(g, res, qdt):
    """Pre-differenced quad table: per cell [g12-g11, g22-g21, g11, g21]."""
    g2 = np.asarray(g, dtype=np.float32).reshape(res, res, FEAT)
    q = np.zeros((res, res, 4, FEAT), dtype=qdt)
    g11 = g2[:res - 1, :res - 1]
    g12 = g2[:res - 1, 1:]
    g21 = g2[1:, :res - 1]
    g22 = g2[1:, 1:]
    q[:res - 1, :res - 1, 0] = g12 - g11
    q[:res - 1, :res - 1, 1] = g22 - g21
    q[:res - 1, :res - 1, 2] = g11
    q[:res - 1, :res - 1, 3] = g21
    return q.reshape(res * res, 16)


def kernel(**inputs):
    x = np.asarray(inputs["x"], dtype=np.float32)
    key = QUAD_DT
    if key not in _cache:
        _cache[key] = _build_program(QUAD_DT)
    nc, names = _cache[key]

    quads = [_quad_table(inputs[f"grid_{i}"], res, QUAD_DT)
             for i, res in enumerate(LODS)]

    total = N_CORES * PTS_PER_CORE
    x_pad = np.full((total, 2), 0.5, dtype=np.float32)
    x_pad[:N_POINTS] = x
    x_sh = x_pad.reshape(N_CORES, P, PPP, 2).reshape(N_CORES, P, PPP * 2)

    in_maps = []
    for c in range(N_CORES):
        m = {names["x"]: x_sh[c]}
        for l in range(NUM_LODS):
            m[names["q"][l]] = quads[l]
        in_maps.append(m)

    res = run_bass_kernel_spmd(nc, in_maps, core_ids=list(range(N_CORES)))
    out = np.empty((total, NUM_LODS * FEAT), dtype=np.float32)
    for c in range(N_CORES):
        blk = res.results[c][names["out"]]
        out[c * PTS_PER_CORE:(c + 1) * PTS_PER_CORE] = blk.reshape(
            P * PPP, NUM_LODS * FEAT)
    return out[:N_POINTS]


# revision 29
# speedup vs baseline: 1.3341x; 1.3341x over previous
"""DenseGrid multi-LOD bilinear embedding lookup on 8 Trainium2 NeuronCores.

Strategy: data-parallel over points (8-way shard). Grids are host-expanded
into per-cell "quad" tables holding the 4 bilinear corners in pre-differenced
form [g12-g11, g22-g21, g11, g21] so the device does a single 64B (fp32) /
32B (fp16) indirect-DMA gather per (point, LOD) and a 2-stage Horner lerp:
    r_i = g_i1 + fx * d_i          (i = rows y1, y1+1)
    out = r_1 + fy * (r_2 - r_1)
"""
import numpy as np
import concourse.bacc as bacc
import concourse.bass as bass
import concourse.mybir as mybir
import concourse.tile as tile
from concourse.bass_utils import run_bass_kernel_spmd

BASE_LOD = 4
NUM_LODS = 8
FEAT = 4
LODS = [2 ** L for L in range(BASE_LOD, BASE_LOD + NUM_LODS)]
N_POINTS = 2_000_000
N_CORES = 8
P = 128
PPP = 2048            # points per partition (per core)
CN = 64               # points per partition per chunk
CHUNKS = PPP // CN
PTS_PER_CORE = P * PPP

QUAD_DT = np.float16  # table dtype (fp32 also supported)
_BIR_QDT = {np.float16: mybir.dt.float16, np.float32: mybir.dt.float32}

# LODs gathered via GPSIMD ap_gather (3.4 ns/point) instead of per-partition
# indirect DMA (11 ns/point). Processed fully channel-major; host unscrambles.
AP_LODS = [0, 1, 2]
NAP = len(AP_LODS)

_cache = {}


def _plane_table(g, res):
    """[128, res*res] f32 container of packed fp16 (d, glo) words.

    Channel role c = p % 16: c < 8 -> (dy, f) = (c // 4, c % 4);
    c >= 8 -> dy-swapped duplicate (1 - (c-8)//4, (c-8) % 4).
    word(cell y,x) = (G_f[y+dy, x+1] - G_f[y+dy, x], G_f[y+dy, x]) fp16 pair.
    """
    g2 = np.asarray(g, dtype=np.float32).reshape(res, res, FEAT)
    planes = {}
    for dy in range(2):
        for f in range(FEAT):
            pl = np.zeros((res, res, 2), dtype=np.float16)
            rows = g2[dy:res - 1 + dy, :, f]
            pl[:res - 1, :res - 1, 1] = rows[:, :res - 1]
            pl[:res - 1, :res - 1, 0] = (
                rows[:, 1:].astype(np.float32)
                - rows[:, :res - 1].astype(np.float32)).astype(np.float16)
            planes[(dy, f)] = pl.reshape(res * res, 2)
    out = np.zeros((128, res * res, 2), dtype=np.float16)
    for p in range(128):
        c = p % 16
        dy, f = (c // 4, c % 4) if c < 8 else (1 - (c - 8) // 4, (c - 8) % 4)
        out[p] = planes[(dy, f)]
    return np.ascontiguousarray(out).view(np.float32).reshape(128, res * res)


def _build_program(qdt):
    bir_qdt = _BIR_QDT[qdt]
    nc = bacc.Bacc(None, target_bir_lowering=False)
    with tile.TileContext(nc) as tc:
        with tc.tile_pool(name="dram", bufs=1, space="DRAM") as dram, \
             tc.tile_pool(name="io", bufs=2) as io, \
             tc.tile_pool(name="qp", bufs=3) as qp, \
             tc.tile_pool(name="pp", bufs=1) as pp, \
             tc.tile_pool(name="vv", bufs=2) as vv, \
             tc.tile_pool(name="cm", bufs=1) as cm, \
             tc.tile_pool(name="wk", bufs=2) as wk:
            x_d = dram.tile([P, PPP * 2], mybir.dt.float32, kind="ExternalInput")
            q_d = [dram.tile([res * res, 16], bir_qdt, kind="ExternalInput",
                             name=f"quad_{li}")
                   for li, res in enumerate(LODS)]
            pl_d = [dram.tile([P, LODS[l] * LODS[l]], mybir.dt.float32,
                              kind="ExternalInput", name=f"plane_{l}")
                    for l in AP_LODS]
            xcm_d = dram.tile([P, 16 * PPP * 2], mybir.dt.float32,
                              kind="ExternalInput")
            out_d = dram.tile([P, PPP * NUM_LODS * FEAT], mybir.dt.float32,
                              kind="ExternalOutput")
            oap_d = [dram.tile([P, 16 * PPP], mybir.dt.float32,
                               kind="ExternalOutput", name=f"oap_{l}")
                     for l in AP_LODS]

            pl_t = []
            for li, l in enumerate(AP_LODS):
                plt = pp.tile([P, LODS[l] * LODS[l]], mybir.dt.float32,
                              tag=f"plt{l}", name=f"plt_{l}")
                nc.sync.dma_start(out=plt[:], in_=pl_d[li][:])
                pl_t.append(plt)

            J = 16 * CN  # channel-major stream length per chunk
            for c in range(CHUNKS):
                xt = io.tile([P, CN * 2], mybir.dt.float32, tag="x")
                nc.sync.dma_start(out=xt[:], in_=x_d[:, c * CN * 2:(c + 1) * CN * 2])
                x3 = xt[:].rearrange("p (n two) -> p n two", two=2)
                ot = io.tile([P, CN * NUM_LODS * FEAT], mybir.dt.float32, tag="o")
                o3 = ot[:].rearrange("p (n f) -> p n f", f=NUM_LODS * FEAT)

                # ---- channel-major fractions for ap_gather LODs ----
                xcm = cm.tile([P, J * 2], mybir.dt.float32, tag="xcm")
                nc.sync.dma_start(out=xcm[:],
                                  in_=xcm_d[:, c * J * 2:(c + 1) * J * 2])
                xcm3 = xcm[:].rearrange("p (j two) -> p j two", two=2)

                def cm_frac(res, coord, tagp):
                    ss = cm.tile([P, J], mybir.dt.float32, tag=f"ss{tagp}")
                    nc.scalar.activation(out=ss[:], in_=xcm3[:, :, coord],
                                         func=mybir.ActivationFunctionType.Copy,
                                         scale=float(res - 1))
                    sc = cm.tile([P, J], mybir.dt.float32, tag=f"sc{tagp}")
                    nc.vector.tensor_scalar(
                        out=sc[:], in0=ss[:], scalar1=float(res - 1) - 1e-5,
                        scalar2=0.5, op0=mybir.AluOpType.min,
                        op1=mybir.AluOpType.subtract)
                    si = cm.tile([P, J], mybir.dt.int32, tag=f"si{tagp}")
                    nc.vector.tensor_copy(out=si[:], in_=sc[:])
                    sf = cm.tile([P, J], mybir.dt.float32, tag=f"sf{tagp}")
                    nc.vector.tensor_copy(out=sf[:], in_=si[:])
                    fr = cm.tile([P, J], mybir.dt.float32, tag=f"fr{tagp}")
                    nc.vector.tensor_sub(out=fr[:], in0=ss[:], in1=sf[:])
                    return fr

                for l, res in enumerate(LODS):
                    if l in AP_LODS:
                        li = AP_LODS.index(l)
                        fxc = cm_frac(res, 0, "x")
                        fyc = cm_frac(res, 1, "y")
                        # idx (point-major wrapped == ap_gather layout)
                        xs = wk.tile([P, CN], mybir.dt.float32, tag="xs")
                        ys = wk.tile([P, CN], mybir.dt.float32, tag="ys")
                        nc.scalar.activation(
                            out=xs[:], in_=x3[:, :, 0],
                            func=mybir.ActivationFunctionType.Copy,
                            scale=float(res - 1))
                        nc.scalar.activation(
                            out=ys[:], in_=x3[:, :, 1],
                            func=mybir.ActivationFunctionType.Copy,
                            scale=float(res - 1))
                        hi = float(res - 1) - 1e-5
                        xc2 = wk.tile([P, CN], mybir.dt.float32, tag="xc")
                        yc2 = wk.tile([P, CN], mybir.dt.float32, tag="yc")
                        nc.vector.tensor_scalar(
                            out=xc2[:], in0=xs[:], scalar1=hi, scalar2=0.5,
                            op0=mybir.AluOpType.min, op1=mybir.AluOpType.subtract)
                        nc.vector.tensor_scalar(
                            out=yc2[:], in0=ys[:], scalar1=hi, scalar2=0.5,
                            op0=mybir.AluOpType.min, op1=mybir.AluOpType.subtract)
                        x1i = wk.tile([P, CN], mybir.dt.int32, tag="x1i")
                        y1i = wk.tile([P, CN], mybir.dt.int32, tag="y1i")
                        nc.vector.tensor_copy(out=x1i[:], in_=xc2[:])
                        nc.vector.tensor_copy(out=y1i[:], in_=yc2[:])
                        x1f = wk.tile([P, CN], mybir.dt.float32, tag="x1f")
                        y1f = wk.tile([P, CN], mybir.dt.float32, tag="y1f")
                        nc.vector.tensor_copy(out=x1f[:], in_=x1i[:])
                        nc.vector.tensor_copy(out=y1f[:], in_=y1i[:])
                        idf = wk.tile([P, CN], mybir.dt.float32, tag="idf")
                        nc.vector.scalar_tensor_tensor(
                            out=idf[:], in0=y1f[:], scalar=float(res),
                            in1=x1f[:], op0=mybir.AluOpType.mult,
                            op1=mybir.AluOpType.add)
                        idx16 = wk.tile([P, CN], mybir.dt.int16, tag="idx16")
                        nc.vector.tensor_copy(out=idx16[:], in_=idf[:])

                        v = vv.tile([P, J], mybir.dt.float32, tag="v")
                        nc.gpsimd.ap_gather(
                            out_ap=v[:], in_ap=pl_t[li][:], idxs_ap=idx16[:],
                            channels=P, num_elems=res * res, d=1, num_idxs=J)

                        # channel-major Horner combine
                        vh = v[:].bitcast(mybir.dt.float16).rearrange(
                            "p (j two) -> p j two", two=2)
                        mm = cm.tile([P, J], mybir.dt.float32, tag="mm")
                        nc.vector.tensor_mul(out=mm[:], in0=vh[:, :, 0],
                                             in1=fxc[:])
                        rr = cm.tile([P, J], mybir.dt.float32, tag="rr")
                        nc.vector.tensor_add(out=rr[:], in0=mm[:],
                                             in1=vh[:, :, 1])
                        rsh = cm.tile([P, J], mybir.dt.float32, tag="rsh")
                        nc.sync.dma_start(out=rsh[:][0:120, :],
                                          in_=rr[:][8:128, :])
                        dyt = cm.tile([P, J], mybir.dt.float32, tag="dyt")
                        nc.vector.tensor_sub(out=dyt[:][0:120, :],
                                             in0=rsh[:][0:120, :],
                                             in1=rr[:][0:120, :])
                        myt = cm.tile([P, J], mybir.dt.float32, tag="myt")
                        nc.vector.tensor_mul(out=myt[:][0:120, :],
                                             in0=dyt[:][0:120, :],
                                             in1=fyc[:][0:120, :])
                        oc = cm.tile([P, J], mybir.dt.float32, tag="oc")
                        nc.vector.tensor_add(out=oc[:][0:120, :],
                                             in0=myt[:][0:120, :],
                                             in1=rr[:][0:120, :])
                        nc.sync.dma_start(
                            out=oap_d[li][:][0:120, c * J:(c + 1) * J],
                            in_=oc[:][0:120, :])
                        continue
                    # --- index / fraction compute ---
                    xs = wk.tile([P, CN], mybir.dt.float32, tag="xs")
                    ys = wk.tile([P, CN], mybir.dt.float32, tag="ys")
                    nc.scalar.activation(out=xs[:], in_=x3[:, :, 0],
                                         func=mybir.ActivationFunctionType.Copy,
                                         scale=float(res - 1))
                    nc.scalar.activation(out=ys[:], in_=x3[:, :, 1],
                                         func=mybir.ActivationFunctionType.Copy,
                                         scale=float(res - 1))
                    # floor via round-nearest int convert of (min(xs,hi) - 0.5);
                    # ties land only on exact-integer coords where the lerp
                    # result is unchanged (fx becomes 1.0 instead of 0.0).
                    hi = float(res - 1) - 1e-5
                    xc = wk.tile([P, CN], mybir.dt.float32, tag="xc")
                    yc = wk.tile([P, CN], mybir.dt.float32, tag="yc")
                    nc.vector.tensor_scalar(out=xc[:], in0=xs[:], scalar1=hi,
                                            scalar2=0.5, op0=mybir.AluOpType.min,
                                            op1=mybir.AluOpType.subtract)
                    nc.vector.tensor_scalar(out=yc[:], in0=ys[:], scalar1=hi,
                                            scalar2=0.5, op0=mybir.AluOpType.min,
                                            op1=mybir.AluOpType.subtract)
                    x1i = wk.tile([P, CN], mybir.dt.int32, tag="x1i")
                    y1i = wk.tile([P, CN], mybir.dt.int32, tag="y1i")
                    nc.vector.tensor_copy(out=x1i[:], in_=xc[:])
                    nc.vector.tensor_copy(out=y1i[:], in_=yc[:])
                    x1f = wk.tile([P, CN], mybir.dt.float32, tag="x1f")
                    y1f = wk.tile([P, CN], mybir.dt.float32, tag="y1f")
                    nc.vector.tensor_copy(out=x1f[:], in_=x1i[:])
                    nc.vector.tensor_copy(out=y1f[:], in_=y1i[:])
                    fx = wk.tile([P, CN], mybir.dt.float32, tag="fx")
                    fy = wk.tile([P, CN], mybir.dt.float32, tag="fy")
                    nc.vector.tensor_sub(out=fx[:], in0=xs[:], in1=x1f[:])
                    nc.vector.tensor_sub(out=fy[:], in0=ys[:], in1=y1f[:])
                    idf = wk.tile([P, CN], mybir.dt.float32, tag="idf")
                    nc.vector.scalar_tensor_tensor(
                        out=idf[:], in0=y1f[:], scalar=float(res), in1=x1f[:],
                        op0=mybir.AluOpType.mult, op1=mybir.AluOpType.add)
                    idx = wk.tile([P, CN], mybir.dt.int32, tag="idx")
                    nc.vector.tensor_copy(out=idx[:], in_=idf[:])

                    # --- gather quads ---
                    # HW indirect DMA uses ONE offset per partition, so issue
                    # one call per point-column (each gathers 128 quads).
                    qt = qp.tile([P, CN * 16], bir_qdt, tag="q")
                    for j in range(CN):
                        nc.gpsimd.indirect_dma_start(
                            out=qt[:, j * 16:(j + 1) * 16], out_offset=None,
                            in_=q_d[l][:],
                            in_offset=bass.IndirectOffsetOnAxis(
                                ap=idx[:, j:j + 1], axis=0))
                    q4 = qt[:].rearrange("p (n c f) -> p n c f", c=4, f=4)

                    # --- Horner bilinear combine ---
                    # quad layout: [d1, d2, g11, g21]
                    fxb = fx[:].unsqueeze(2).unsqueeze(3).broadcast_to([P, CN, 2, 4])
                    m = wk.tile([P, CN * 8], mybir.dt.float32, tag="m")
                    m4 = m[:].rearrange("p (n c f) -> p n c f", c=2, f=4)
                    nc.vector.tensor_mul(out=m4, in0=q4[:, :, 0:2, :], in1=fxb)
                    r = wk.tile([P, CN * 8], mybir.dt.float32, tag="r")
                    r4 = r[:].rearrange("p (n c f) -> p n c f", c=2, f=4)
                    nc.vector.tensor_add(out=r4, in0=m4, in1=q4[:, :, 2:4, :])
                    dy = wk.tile([P, CN * 4], mybir.dt.float32, tag="dy")
                    dy3 = dy[:].rearrange("p (n f) -> p n f", f=4)
                    nc.vector.tensor_sub(out=dy3, in0=r4[:, :, 1, :], in1=r4[:, :, 0, :])
                    fyb = fy[:].unsqueeze(2).broadcast_to([P, CN, 4])
                    my = wk.tile([P, CN * 4], mybir.dt.float32, tag="my")
                    my3 = my[:].rearrange("p (n f) -> p n f", f=4)
                    nc.vector.tensor_mul(out=my3, in0=dy3, in1=fyb)
                    nc.vector.tensor_add(out=o3[:, :, l * FEAT:(l + 1) * FEAT],
                                         in0=my3, in1=r4[:, :, 0, :])

                nc.sync.dma_start(
                    out=out_d[:, c * CN * NUM_LODS * FEAT:(c + 1) * CN * NUM_LODS * FEAT],
                    in_=ot[:])
    nc.compile()
    names = {"x": x_d.name, "q": [t.name for t in q_d],
             "pl": [t.name for t in pl_d], "xcm": xcm_d.name,
             "oap": [t.name for t in oap_d], "out": out_d.name}
    return nc, names


def _quad_table(g, res, qdt):
    """Pre-differenced quad table: per cell [g12-g11, g22-g21, g11, g21]."""
    g2 = np.asarray(g, dtype=np.float32).reshape(res, res, FEAT)
    q = np.zeros((res, res, 4, FEAT), dtype=qdt)
    g11 = g2[:res - 1, :res - 1]
    g12 = g2[:res - 1, 1:]
    g21 = g2[1:, :res - 1]
    g22 = g2[1:, 1:]
    q[:res - 1, :res - 1, 0] = g12 - g11
    q[:res - 1, :res - 1, 1] = g22 - g21
    q[:res - 1, :res - 1, 2] = g11
    q[:res - 1, :res - 1, 3] = g21
    return q.reshape(res * res, 16)


def kernel(**inputs):
    x = np.asarray(inputs["x"], dtype=np.float32)
    assert x.shape == (N_POINTS, 2), x.shape
    key = QUAD_DT
    if key not in _cache:
        _cache[key] = _build_program(QUAD_DT)
    nc, names = _cache[key]

    quads = [_quad_table(inputs[f"grid_{i}"], res, QUAD_DT)
             for i, res in enumerate(LODS)]

    planes = [_plane_table(inputs[f"grid_{l}"], LODS[l]) for l in AP_LODS]

    total = N_CORES * PTS_PER_CORE
    x_pad = np.full((total, 2), 0.5, dtype=np.float32)
    x_pad[:N_POINTS] = x
    x_sh = x_pad.reshape(N_CORES, P, PPP, 2).reshape(N_CORES, P, PPP * 2)
    # channel-major replicated x: xcm[16g+c, j=(n,i)] = x(point(16g+i, n))
    x5 = x_pad.reshape(N_CORES, 8, 16, PPP, 2)          # (core, g, i, n, 2)
    xcm = np.ascontiguousarray(x5.transpose(0, 1, 3, 2, 4))  # (core, g, n, i, 2)
    xcm = xcm.reshape(N_CORES, 8, 1, 16 * PPP * 2)
    xcm = np.broadcast_to(xcm, (N_CORES, 8, 16, 16 * PPP * 2))
    xcm = np.ascontiguousarray(xcm).reshape(N_CORES, P, 16 * PPP * 2)

    in_maps = []
    for c in range(N_CORES):
        m = {names["x"]: x_sh[c], names["xcm"]: xcm[c]}
        for l in range(NUM_LODS):
            m[names["q"][l]] = quads[l]
        for li in range(NAP):
            m[names["pl"][li]] = planes[li]
        in_maps.append(m)

    res = run_bass_kernel_spmd(nc, in_maps, core_ids=list(range(N_CORES)))
    out = np.empty((total, NUM_LODS * FEAT), dtype=np.float32)
    for c in range(N_CORES):
        blk = np.array(res.results[c][names["out"]]).reshape(
            P * PPP, NUM_LODS * FEAT)
        for li, l in enumerate(AP_LODS):
            a = np.asarray(res.results[c][names["oap"][li]])
            a = a.reshape(8, 16, PPP, 16)[:, :FEAT]      # (g, f, n, i)
            a = a.transpose(0, 3, 2, 1).reshape(P * PPP, FEAT)  # (g,i,n,f)
            blk[:, l * FEAT:(l + 1) * FEAT] = a
        out[c * PTS_PER_CORE:(c + 1) * PTS_PER_CORE] = blk
    return out[:N_POINTS]


# revision 30
# speedup vs baseline: 1.4526x; 1.0888x over previous
"""DenseGrid multi-LOD bilinear embedding lookup on 8 Trainium2 NeuronCores.

Strategy: data-parallel over points (8-way shard). Grids are host-expanded
into per-cell "quad" tables holding the 4 bilinear corners in pre-differenced
form [g12-g11, g22-g21, g11, g21] so the device does a single 64B (fp32) /
32B (fp16) indirect-DMA gather per (point, LOD) and a 2-stage Horner lerp:
    r_i = g_i1 + fx * d_i          (i = rows y1, y1+1)
    out = r_1 + fy * (r_2 - r_1)
"""
import numpy as np
import concourse.bacc as bacc
import concourse.bass as bass
import concourse.mybir as mybir
import concourse.tile as tile
from concourse.bass_utils import run_bass_kernel_spmd

BASE_LOD = 4
NUM_LODS = 8
FEAT = 4
LODS = [2 ** L for L in range(BASE_LOD, BASE_LOD + NUM_LODS)]
N_POINTS = 2_000_000
N_CORES = 8
P = 128
PPP = 2048            # points per partition (per core)
CN = 64               # points per partition per chunk
CHUNKS = PPP // CN
PTS_PER_CORE = P * PPP

QUAD_DT = np.float16  # table dtype (fp32 also supported)
_BIR_QDT = {np.float16: mybir.dt.float16, np.float32: mybir.dt.float32}

# LODs gathered via GPSIMD ap_gather (3.4 ns/point) instead of per-partition
# indirect DMA (11 ns/point). Processed fully channel-major; host unscrambles.
AP_LODS = [0, 1, 2, 3]
NAP = len(AP_LODS)

_cache = {}


def _plane_table(g, res):
    """[128, res*res] f32 container of packed fp16 (d, glo) words.

    Channel role c = p % 16: c < 8 -> (dy, f) = (c // 4, c % 4);
    c >= 8 -> dy-swapped duplicate (1 - (c-8)//4, (c-8) % 4).
    word(cell y,x) = (G_f[y+dy, x+1] - G_f[y+dy, x], G_f[y+dy, x]) fp16 pair.
    """
    g2 = np.asarray(g, dtype=np.float32).reshape(res, res, FEAT)
    planes = {}
    for dy in range(2):
        for f in range(FEAT):
            pl = np.zeros((res, res, 2), dtype=np.float16)
            rows = g2[dy:res - 1 + dy, :, f]
            pl[:res - 1, :res - 1, 1] = rows[:, :res - 1]
            pl[:res - 1, :res - 1, 0] = (
                rows[:, 1:].astype(np.float32)
                - rows[:, :res - 1].astype(np.float32)).astype(np.float16)
            planes[(dy, f)] = pl.reshape(res * res, 2)
    out = np.zeros((128, res * res, 2), dtype=np.float16)
    for p in range(128):
        c = p % 16
        dy, f = (c // 4, c % 4) if c < 8 else (1 - (c - 8) // 4, (c - 8) % 4)
        out[p] = planes[(dy, f)]
    return np.ascontiguousarray(out).view(np.float32).reshape(128, res * res)


def _build_program(qdt):
    bir_qdt = _BIR_QDT[qdt]
    nc = bacc.Bacc(None, target_bir_lowering=False)
    with tile.TileContext(nc) as tc:
        with tc.tile_pool(name="dram", bufs=1, space="DRAM") as dram, \
             tc.tile_pool(name="io", bufs=2) as io, \
             tc.tile_pool(name="qp", bufs=3) as qp, \
             tc.tile_pool(name="pp", bufs=1) as pp, \
             tc.tile_pool(name="vv", bufs=2) as vv, \
             tc.tile_pool(name="cm", bufs=1) as cm, \
             tc.tile_pool(name="wk", bufs=2) as wk:
            x_d = dram.tile([P, PPP * 2], mybir.dt.float32, kind="ExternalInput")
            q_d = [dram.tile([res * res, 16], bir_qdt, kind="ExternalInput",
                             name=f"quad_{li}")
                   for li, res in enumerate(LODS)]
            pl_d = [dram.tile([P, LODS[l] * LODS[l]], mybir.dt.float32,
                              kind="ExternalInput", name=f"plane_{l}")
                    for l in AP_LODS]
            xcm_d = dram.tile([P, 16 * PPP * 2], mybir.dt.float32,
                              kind="ExternalInput")
            out_d = dram.tile([P, PPP * NUM_LODS * FEAT], mybir.dt.float32,
                              kind="ExternalOutput")
            oap_d = [dram.tile([P, 16 * PPP], mybir.dt.float32,
                               kind="ExternalOutput", name=f"oap_{l}")
                     for l in AP_LODS]

            pl_t = []
            for li, l in enumerate(AP_LODS):
                plt = pp.tile([P, LODS[l] * LODS[l]], mybir.dt.float32,
                              tag=f"plt{l}", name=f"plt_{l}")
                nc.sync.dma_start(out=plt[:], in_=pl_d[li][:])
                pl_t.append(plt)

            J = 16 * CN  # channel-major stream length per chunk
            for c in range(CHUNKS):
                xt = io.tile([P, CN * 2], mybir.dt.float32, tag="x")
                nc.sync.dma_start(out=xt[:], in_=x_d[:, c * CN * 2:(c + 1) * CN * 2])
                x3 = xt[:].rearrange("p (n two) -> p n two", two=2)
                ot = io.tile([P, CN * NUM_LODS * FEAT], mybir.dt.float32, tag="o")
                o3 = ot[:].rearrange("p (n f) -> p n f", f=NUM_LODS * FEAT)

                # ---- channel-major fractions for ap_gather LODs ----
                xcm = cm.tile([P, J * 2], mybir.dt.float32, tag="xcm")
                nc.sync.dma_start(out=xcm[:],
                                  in_=xcm_d[:, c * J * 2:(c + 1) * J * 2])
                xcm3 = xcm[:].rearrange("p (j two) -> p j two", two=2)

                def cm_frac(res, coord, tagp):
                    # scratch tags shared across coords (bufs=1 serializes)
                    ss = cm.tile([P, J], mybir.dt.float32, tag="ss")
                    nc.scalar.activation(out=ss[:], in_=xcm3[:, :, coord],
                                         func=mybir.ActivationFunctionType.Copy,
                                         scale=float(res - 1))
                    sc = cm.tile([P, J], mybir.dt.float32, tag="sc")
                    nc.vector.tensor_scalar(
                        out=sc[:], in0=ss[:], scalar1=float(res - 1) - 1e-5,
                        scalar2=0.5, op0=mybir.AluOpType.min,
                        op1=mybir.AluOpType.subtract)
                    si = cm.tile([P, J], mybir.dt.int32, tag="si")
                    nc.vector.tensor_copy(out=si[:], in_=sc[:])
                    sf = cm.tile([P, J], mybir.dt.float32, tag="sf")
                    nc.vector.tensor_copy(out=sf[:], in_=si[:])
                    fr = cm.tile([P, J], mybir.dt.float32, tag=f"fr{tagp}")
                    nc.vector.tensor_sub(out=fr[:], in0=ss[:], in1=sf[:])
                    return fr

                for l, res in enumerate(LODS):
                    if l in AP_LODS:
                        li = AP_LODS.index(l)
                        fxc = cm_frac(res, 0, "x")
                        fyc = cm_frac(res, 1, "y")
                        # idx (point-major wrapped == ap_gather layout)
                        xs = wk.tile([P, CN], mybir.dt.float32, tag="xs")
                        ys = wk.tile([P, CN], mybir.dt.float32, tag="ys")
                        nc.scalar.activation(
                            out=xs[:], in_=x3[:, :, 0],
                            func=mybir.ActivationFunctionType.Copy,
                            scale=float(res - 1))
                        nc.scalar.activation(
                            out=ys[:], in_=x3[:, :, 1],
                            func=mybir.ActivationFunctionType.Copy,
                            scale=float(res - 1))
                        hi = float(res - 1) - 1e-5
                        xc2 = wk.tile([P, CN], mybir.dt.float32, tag="xc")
                        yc2 = wk.tile([P, CN], mybir.dt.float32, tag="yc")
                        nc.vector.tensor_scalar(
                            out=xc2[:], in0=xs[:], scalar1=hi, scalar2=0.5,
                            op0=mybir.AluOpType.min, op1=mybir.AluOpType.subtract)
                        nc.vector.tensor_scalar(
                            out=yc2[:], in0=ys[:], scalar1=hi, scalar2=0.5,
                            op0=mybir.AluOpType.min, op1=mybir.AluOpType.subtract)
                        x1i = wk.tile([P, CN], mybir.dt.int32, tag="x1i")
                        y1i = wk.tile([P, CN], mybir.dt.int32, tag="y1i")
                        nc.vector.tensor_copy(out=x1i[:], in_=xc2[:])
                        nc.vector.tensor_copy(out=y1i[:], in_=yc2[:])
                        x1f = wk.tile([P, CN], mybir.dt.float32, tag="x1f")
                        y1f = wk.tile([P, CN], mybir.dt.float32, tag="y1f")
                        nc.vector.tensor_copy(out=x1f[:], in_=x1i[:])
                        nc.vector.tensor_copy(out=y1f[:], in_=y1i[:])
                        idf = wk.tile([P, CN], mybir.dt.float32, tag="idf")
                        nc.vector.scalar_tensor_tensor(
                            out=idf[:], in0=y1f[:], scalar=float(res),
                            in1=x1f[:], op0=mybir.AluOpType.mult,
                            op1=mybir.AluOpType.add)
                        idx16 = wk.tile([P, CN], mybir.dt.int16, tag="idx16")
                        nc.vector.tensor_copy(out=idx16[:], in_=idf[:])

                        v = vv.tile([P, J], mybir.dt.float32, tag="v")
                        nc.gpsimd.ap_gather(
                            out_ap=v[:], in_ap=pl_t[li][:], idxs_ap=idx16[:],
                            channels=P, num_elems=res * res, d=1, num_idxs=J)

                        # channel-major Horner combine
                        vh = v[:].bitcast(mybir.dt.float16).rearrange(
                            "p (j two) -> p j two", two=2)
                        mm = cm.tile([P, J], mybir.dt.float32, tag="mm")
                        nc.vector.tensor_mul(out=mm[:], in0=vh[:, :, 0],
                                             in1=fxc[:])
                        rr = cm.tile([P, J], mybir.dt.float32, tag="rr")
                        nc.vector.tensor_add(out=rr[:], in0=mm[:],
                                             in1=vh[:, :, 1])
                        rsh = cm.tile([P, J], mybir.dt.float32, tag="rsh")
                        nc.sync.dma_start(out=rsh[:][0:120, :],
                                          in_=rr[:][8:128, :])
                        dyt = cm.tile([P, J], mybir.dt.float32, tag="dyt")
                        nc.vector.tensor_sub(out=dyt[:][0:120, :],
                                             in0=rsh[:][0:120, :],
                                             in1=rr[:][0:120, :])
                        myt = cm.tile([P, J], mybir.dt.float32, tag="myt")
                        nc.vector.tensor_mul(out=myt[:][0:120, :],
                                             in0=dyt[:][0:120, :],
                                             in1=fyc[:][0:120, :])
                        oc = cm.tile([P, J], mybir.dt.float32, tag="oc")
                        nc.vector.tensor_add(out=oc[:][0:120, :],
                                             in0=myt[:][0:120, :],
                                             in1=rr[:][0:120, :])
                        nc.sync.dma_start(
                            out=oap_d[li][:][0:120, c * J:(c + 1) * J],
                            in_=oc[:][0:120, :])
                        continue
                    # --- index / fraction compute ---
                    xs = wk.tile([P, CN], mybir.dt.float32, tag="xs")
                    ys = wk.tile([P, CN], mybir.dt.float32, tag="ys")
                    nc.scalar.activation(out=xs[:], in_=x3[:, :, 0],
                                         func=mybir.ActivationFunctionType.Copy,
                                         scale=float(res - 1))
                    nc.scalar.activation(out=ys[:], in_=x3[:, :, 1],
                                         func=mybir.ActivationFunctionType.Copy,
                                         scale=float(res - 1))
                    # floor via round-nearest int convert of (min(xs,hi) - 0.5);
                    # ties land only on exact-integer coords where the lerp
                    # result is unchanged (fx becomes 1.0 instead of 0.0).
                    hi = float(res - 1) - 1e-5
                    xc = wk.tile([P, CN], mybir.dt.float32, tag="xc")
                    yc = wk.tile([P, CN], mybir.dt.float32, tag="yc")
                    nc.vector.tensor_scalar(out=xc[:], in0=xs[:], scalar1=hi,
                                            scalar2=0.5, op0=mybir.AluOpType.min,
                                            op1=mybir.AluOpType.subtract)
                    nc.vector.tensor_scalar(out=yc[:], in0=ys[:], scalar1=hi,
                                            scalar2=0.5, op0=mybir.AluOpType.min,
                                            op1=mybir.AluOpType.subtract)
                    x1i = wk.tile([P, CN], mybir.dt.int32, tag="x1i")
                    y1i = wk.tile([P, CN], mybir.dt.int32, tag="y1i")
                    nc.vector.tensor_copy(out=x1i[:], in_=xc[:])
                    nc.vector.tensor_copy(out=y1i[:], in_=yc[:])
                    x1f = wk.tile([P, CN], mybir.dt.float32, tag="x1f")
                    y1f = wk.tile([P, CN], mybir.dt.float32, tag="y1f")
                    nc.vector.tensor_copy(out=x1f[:], in_=x1i[:])
                    nc.vector.tensor_copy(out=y1f[:], in_=y1i[:])
                    fx = wk.tile([P, CN], mybir.dt.float32, tag="fx")
                    fy = wk.tile([P, CN], mybir.dt.float32, tag="fy")
                    nc.vector.tensor_sub(out=fx[:], in0=xs[:], in1=x1f[:])
                    nc.vector.tensor_sub(out=fy[:], in0=ys[:], in1=y1f[:])
                    idf = wk.tile([P, CN], mybir.dt.float32, tag="idf")
                    nc.vector.scalar_tensor_tensor(
                        out=idf[:], in0=y1f[:], scalar=float(res), in1=x1f[:],
                        op0=mybir.AluOpType.mult, op1=mybir.AluOpType.add)
                    idx = wk.tile([P, CN], mybir.dt.int32, tag="idx")
                    nc.vector.tensor_copy(out=idx[:], in_=idf[:])

                    # --- gather quads ---
                    # HW indirect DMA uses ONE offset per partition, so issue
                    # one call per point-column (each gathers 128 quads).
                    qt = qp.tile([P, CN * 16], bir_qdt, tag="q")
                    for j in range(CN):
                        nc.gpsimd.indirect_dma_start(
                            out=qt[:, j * 16:(j + 1) * 16], out_offset=None,
                            in_=q_d[l][:],
                            in_offset=bass.IndirectOffsetOnAxis(
                                ap=idx[:, j:j + 1], axis=0))
                    q4 = qt[:].rearrange("p (n c f) -> p n c f", c=4, f=4)

                    # --- Horner bilinear combine ---
                    # quad layout: [d1, d2, g11, g21]
                    fxb = fx[:].unsqueeze(2).unsqueeze(3).broadcast_to([P, CN, 2, 4])
                    m = wk.tile([P, CN * 8], mybir.dt.float32, tag="m")
                    m4 = m[:].rearrange("p (n c f) -> p n c f", c=2, f=4)
                    nc.vector.tensor_mul(out=m4, in0=q4[:, :, 0:2, :], in1=fxb)
                    r = wk.tile([P, CN * 8], mybir.dt.float32, tag="r")
                    r4 = r[:].rearrange("p (n c f) -> p n c f", c=2, f=4)
                    nc.vector.tensor_add(out=r4, in0=m4, in1=q4[:, :, 2:4, :])
                    dy = wk.tile([P, CN * 4], mybir.dt.float32, tag="dy")
                    dy3 = dy[:].rearrange("p (n f) -> p n f", f=4)
                    nc.vector.tensor_sub(out=dy3, in0=r4[:, :, 1, :], in1=r4[:, :, 0, :])
                    fyb = fy[:].unsqueeze(2).broadcast_to([P, CN, 4])
                    my = wk.tile([P, CN * 4], mybir.dt.float32, tag="my")
                    my3 = my[:].rearrange("p (n f) -> p n f", f=4)
                    nc.vector.tensor_mul(out=my3, in0=dy3, in1=fyb)
                    nc.vector.tensor_add(out=o3[:, :, l * FEAT:(l + 1) * FEAT],
                                         in0=my3, in1=r4[:, :, 0, :])

                nc.sync.dma_start(
                    out=out_d[:, c * CN * NUM_LODS * FEAT:(c + 1) * CN * NUM_LODS * FEAT],
                    in_=ot[:])
    nc.compile()
    names = {"x": x_d.name, "q": [t.name for t in q_d],
             "pl": [t.name for t in pl_d], "xcm": xcm_d.name,
             "oap": [t.name for t in oap_d], "out": out_d.name}
    return nc, names


def _quad_table(g, res, qdt):
    """Pre-differenced quad table: per cell [g12-g11, g22-g21, g11, g21]."""
    g2 = np.asarray(g, dtype=np.float32).reshape(res, res, FEAT)
    q = np.zeros((res, res, 4, FEAT), dtype=qdt)
    g11 = g2[:res - 1, :res - 1]
    g12 = g2[:res - 1, 1:]
    g21 = g2[1:, :res - 1]
    g22 = g2[1:, 1:]
    q[:res - 1, :res - 1, 0] = g12 - g11
    q[:res - 1, :res - 1, 1] = g22 - g21
    q[:res - 1, :res - 1, 2] = g11
    q[:res - 1, :res - 1, 3] = g21
    return q.reshape(res * res, 16)


def kernel(**inputs):
    x = np.asarray(inputs["x"], dtype=np.float32)
    assert x.shape == (N_POINTS, 2), x.shape
    key = QUAD_DT
    if key not in _cache:
        _cache[key] = _build_program(QUAD_DT)
    nc, names = _cache[key]

    quads = [_quad_table(inputs[f"grid_{i}"], res, QUAD_DT)
             for i, res in enumerate(LODS)]

    planes = [_plane_table(inputs[f"grid_{l}"], LODS[l]) for l in AP_LODS]

    total = N_CORES * PTS_PER_CORE
    x_pad = np.full((total, 2), 0.5, dtype=np.float32)
    x_pad[:N_POINTS] = x
    x_sh = x_pad.reshape(N_CORES, P, PPP, 2).reshape(N_CORES, P, PPP * 2)
    # channel-major replicated x: xcm[16g+c, j=(n,i)] = x(point(16g+i, n))
    x5 = x_pad.reshape(N_CORES, 8, 16, PPP, 2)          # (core, g, i, n, 2)
    xcm = np.ascontiguousarray(x5.transpose(0, 1, 3, 2, 4))  # (core, g, n, i, 2)
    xcm = xcm.reshape(N_CORES, 8, 1, 16 * PPP * 2)
    xcm = np.broadcast_to(xcm, (N_CORES, 8, 16, 16 * PPP * 2))
    xcm = np.ascontiguousarray(xcm).reshape(N_CORES, P, 16 * PPP * 2)

    in_maps = []
    for c in range(N_CORES):
        m = {names["x"]: x_sh[c], names["xcm"]: xcm[c]}
        for l in range(NUM_LODS):
            m[names["q"][l]] = quads[l]
        for li in range(NAP):
            m[names["pl"][li]] = planes[li]
        in_maps.append(m)

    res = run_bass_kernel_spmd(nc, in_maps, core_ids=list(range(N_CORES)))
    out = np.empty((total, NUM_LODS * FEAT), dtype=np.float32)
    for c in range(N_CORES):
        blk = np.array(res.results[c][names["out"]]).reshape(
            P * PPP, NUM_LODS * FEAT)
        for li, l in enumerate(AP_LODS):
            a = np.asarray(res.results[c][names["oap"][li]])
            a = a.reshape(8, 16, PPP, 16)[:, :FEAT]      # (g, f, n, i)
            a = a.transpose(0, 3, 2, 1).reshape(P * PPP, FEAT)  # (g,i,n,f)
            blk[:, l * FEAT:(l + 1) * FEAT] = a
        out[c * PTS_PER_CORE:(c + 1) * PTS_PER_CORE] = blk
    return out[:N_POINTS]
